# revision 2
# baseline (speedup 1.0000x reference)
"""AnchorLoss distributed Bass kernel for 8 TRN2 NeuronCores.

loss = -(2*n*sum(a^2) - 2*||colsum(a)||^2) / sqrt(dim_emb) / k^2

Strategy (data-parallel over n_classes):
  - Each core streams its [1024, 6144] f32 shard HBM->SBUF in 8 tiles of
    [128, 6144] (3 MiB per DMA, DMA-bound at ~360-420 GB/s).
  - ScalarEngine: Square activation with accum_out -> per-partition local
    sum-of-squares, one pass per tile.
  - TensorEngine: ones-matmuls (one-hot lhsT) accumulate the column-sum of
    all 8 tiles into one PSUM bank laid out as [12, 512].
  - Tiny AllReduce (24.6 KiB: colsum vector + sumsq scalar) across 8 cores.
  - Each core finishes: ||S||^2 via DVE square+reduce plus a ones-matmul,
    then the scalar combine, and writes the identical scalar output.
"""

import math
import sys

import numpy as np

if "/opt/trn_rl_repo" not in sys.path:
    sys.path.insert(0, "/opt/trn_rl_repo")

import concourse.bacc as bacc
import concourse.bass as bass
import concourse.mybir as mybir
import concourse.tile as tile
from concourse.bass_utils import run_bass_kernel_spmd

N_CORES = 8
N_CLASSES = 8192
K_ANCH = 8
DIM_EMB = 768
D = K_ANCH * DIM_EMB           # 6144 features per class row
ROWS = N_CLASSES // N_CORES    # 1024 rows per core
P = 128
N_TILES = ROWS // P            # 8 tiles of [128, D] per core
CHUNK = 512                    # fp32 matmul moving-operand max
N_CHUNKS = D // CHUNK          # 12
CC_LEN = D + 8                 # collective buffer, padded to 32B multiple
F32 = mybir.dt.float32
# loss = COEF * (n*sumsq - ||colsum||^2)
COEF = -2.0 / (math.sqrt(DIM_EMB) * K_ANCH * K_ANCH)


def build():
    nc = bacc.Bacc(
        "TRN2", target_bir_lowering=False, debug=False, num_devices=N_CORES
    )
    a_ext = nc.dram_tensor("anchors", [ROWS, D], F32, kind="ExternalInput")
    out_ext = nc.dram_tensor("out", [1, 1], F32, kind="ExternalOutput")

    oh_np = np.ascontiguousarray(
        np.broadcast_to(
            np.eye(N_CHUNKS, dtype=np.float32), (P, N_CHUNKS, N_CHUNKS)
        )
    )
    oh_dram = nc.inline_tensor(oh_np, name="onehot")
    ones_dram = nc.inline_tensor(np.ones((P, 1), dtype=np.float32), name="ones1")

    with tile.TileContext(nc) as tc:
        with (
            tc.tile_pool(name="inp", bufs=4) as inp_pool,
            tc.tile_pool(name="scr", bufs=1) as scr_pool,
            tc.tile_pool(name="small", bufs=1) as small,
            tc.tile_pool(name="psum", bufs=1, space=bass.MemorySpace.PSUM) as psum_pool,
            tc.tile_pool(name="dram", bufs=1, space=bass.MemorySpace.DRAM) as dram_pool,
        ):
            oh = small.tile([P, N_CHUNKS, N_CHUNKS], F32)
            nc.sync.dma_start(out=oh[:], in_=oh_dram.ap())
            ones1 = small.tile([P, 1], F32)
            nc.sync.dma_start(out=ones1[:], in_=ones_dram.ap())

            sq_parts = small.tile([P, N_TILES], F32)
            scratch = scr_pool.tile([P, D], F32)
            cs_psum = psum_pool.tile([N_CHUNKS, CHUNK], F32)

            a_v = a_ext.ap().rearrange("(t p) d -> t p d", p=P)
            for t in range(N_TILES):
                tl = inp_pool.tile([P, D], F32)
                nc.sync.dma_start(out=tl[:], in_=a_v[t])
                # local sum of squares along the free axis, one col per tile
                nc.scalar.activation(
                    scratch[:],
                    tl[:],
                    mybir.ActivationFunctionType.Square,
                    accum_out=sq_parts[:, t : t + 1],
                )
                # column-sum via one-hot ones-matmuls, all into one PSUM bank
                for j in range(N_CHUNKS):
                    nc.tensor.matmul(
                        cs_psum[:],
                        oh[:, j, :],
                        tl[:, j * CHUNK : (j + 1) * CHUNK],
                        start=(t == 0 and j == 0),
                        stop=(t == N_TILES - 1 and j == N_CHUNKS - 1),
                    )

            # local sum of squares -> scalar in PSUM
            ss_loc = small.tile([P, 1], F32)
            nc.vector.reduce_sum(ss_loc[:], sq_parts[:], axis=mybir.AxisListType.X)
            ss_psum = psum_pool.tile([1, 1], F32)
            nc.tensor.matmul(ss_psum[:], ones1[:], ss_loc[:])

            # stage local partials to DRAM for the collective
            cs_sb = scr_pool.tile([N_CHUNKS, CHUNK], F32)
            nc.vector.tensor_copy(cs_sb[:], cs_psum[:])
            ss_sb = small.tile([1, 1], F32)
            nc.scalar.copy(ss_sb[:], ss_psum[:])

            cc_in = dram_pool.tile([CC_LEN], F32)
            cc_out = dram_pool.tile([CC_LEN], F32)
            nc.sync.dma_start(
                out=cc_in[0:D].rearrange("(r c) -> r c", r=N_CHUNKS), in_=cs_sb[:]
            )
            nc.sync.dma_start(
                out=cc_in[D : D + 1].rearrange("(a b) -> a b", a=1), in_=ss_sb[:]
            )

            nc.gpsimd.collective_compute(
                "AllReduce",
                mybir.AluOpType.add,
                replica_groups=[list(range(N_CORES))],
                ins=[cc_in.opt()],
                outs=[cc_out.opt()],
            )

            # global colsum S laid out [128, 48]; global sumsq scalar
            s48 = small.tile([P, D // P], F32)
            nc.sync.dma_start(
                out=s48[:], in_=cc_out[0:D].rearrange("(p f) -> p f", p=P)
            )
            gss = small.tile([1, 1], F32)
            nc.sync.dma_start(
                out=gss[:], in_=cc_out[D : D + 1].rearrange("(a b) -> a b", a=1)
            )

            # ||S||^2 via Square activation with free-axis accumulate
            sq48 = small.tile([P, D // P], F32)
            dot_p = small.tile([P, 1], F32)
            nc.scalar.activation(
                sq48[:],
                s48[:],
                mybir.ActivationFunctionType.Square,
                accum_out=dot_p[:],
            )
            dot_psum = psum_pool.tile([1, 1], F32)
            nc.tensor.matmul(dot_psum[:], ones1[:], dot_p[:])

            # loss = COEF * (n * gss - ||S||^2)
            a1 = small.tile([1, 1], F32)
            nc.scalar.mul(a1[:], gss[:], float(N_CLASSES))
            d1 = small.tile([1, 1], F32)
            nc.vector.tensor_sub(d1[:], a1[:], dot_psum[:])
            res = small.tile([1, 1], F32)
            nc.scalar.mul(res[:], d1[:], COEF)
            nc.sync.dma_start(out=out_ext.ap(), in_=res[:])

    nc.compile()
    return nc


_NC_CACHE = None


def _get_nc():
    global _NC_CACHE
    if _NC_CACHE is None:
        _NC_CACHE = build()
    return _NC_CACHE


def make_in_maps(anchors: np.ndarray) -> list[dict[str, np.ndarray]]:
    a = np.ascontiguousarray(anchors, dtype=np.float32).reshape(N_CLASSES, D)
    return [
        {"anchors": np.ascontiguousarray(a[c * ROWS : (c + 1) * ROWS])}
        for c in range(N_CORES)
    ]


def kernel(anchors: np.ndarray) -> np.ndarray:
    nc = _get_nc()
    res = run_bass_kernel_spmd(
        nc, make_in_maps(anchors), core_ids=list(range(N_CORES))
    )
    out = np.asarray(res.results[0]["out"], dtype=np.float32)
    return out.reshape(())
